# revision 20
# baseline (speedup 1.0000x reference)
"""DepthAttnLayer Trainium2 kernel: ragged gather-attention over BEV cells.

Strategy (SPMD over 8 cores, one shared program):
  * Host repacks the 32400 ragged BEV cells into 904 uniform "bins" of
    exactly <=36 cells (LPT-balanced so every bin is <= B*128 points),
    113 bins per core; every bin's points padded to B*128 point-slots so
    the device program is identical across cores and bins.
  * Pass 0: k/q in-projections on the PE (bf16); projected-k rows and raw
    value rows are packed side by side into one [SRC, 512] bf16 table so a
    single 1KB-row dma_gather fetches both per point (descriptor count is
    the bottleneck: the Q7 SWDGE generates ~8ns/descriptor).
  * Pass 1 (per bin): dma_gather of kv rows; per-point q is expanded from
    the bin's 36 query rows by a PE matmul with a host-shipped 0/1
    selection matrix S^T (no q gather); per-point q*k head-dot on DVE;
    interval softmax via exp (logits are small, no max-subtract) with the
    per-cell 1/denom applied after the segment reduce; segment reduce back
    to cells with S matmuls on the PE.
  * Pass 2: out-proj + residual + LayerNorm + FFN in 128-row tiles,
    transposing between row-major (LN) and feature-major (matmuls) on PE.
"""
import os
import sys

for _p in ("/opt/trn_rl_repo", "/root/.axon_site/_ro/trn_rl_repo"):
    if os.path.isdir(_p) and _p not in sys.path:
        sys.path.insert(0, _p)

import heapq

import ml_dtypes
import numpy as np

import concourse.bacc as bacc
import concourse.bass as bass
import concourse.mybir as mybir
from concourse import bass_utils
from concourse.masks import make_identity
from concourse.tile import TileContext

F32 = mybir.dt.float32
BF16 = mybir.dt.bfloat16
I16 = mybir.dt.int16
NPBF = ml_dtypes.bfloat16

EMBED = 256
HEADS = 8
HD = 32
TGT = 32400
SRC = 16896
NCORES = 8
CPB = 36                      # cell slots per bin
NBINS = 904                   # total bins (multiple of NCORES)
NB = NBINS // NCORES          # bins per core = 113
SLOTS = NB * CPB              # cell slots per core = 4068
SLOTS_PAD = 4096              # attn/out rows per core (32 tiles of 128)
NT2 = SLOTS_PAD // 128        # pass-2 tiles


def _pack_bins(lengths):
    """LPT-pack cells into NBINS bins of exactly <=CPB slots.

    Returns (bin_of_cell, slot_of_cell)."""
    order = np.argsort(-lengths, kind="stable")
    bin_of = np.empty(TGT, np.int32)
    slot_of = np.empty(TGT, np.int32)
    used = np.zeros(NBINS, np.int32)
    pts = np.zeros(NBINS, np.int64)
    heap = [(0, b) for b in range(NBINS)]
    heapq.heapify(heap)
    for cell in order:
        while True:
            p, b = heapq.heappop(heap)
            if used[b] < CPB and p == pts[b]:
                break
        bin_of[cell] = b
        slot_of[cell] = used[b]
        used[b] += 1
        pts[b] += lengths[cell]
        if used[b] < CPB:
            heapq.heappush(heap, (int(pts[b]), b))
    return bin_of, slot_of


def _host_prep(inputs):
    q_full = np.asarray(inputs["query_depth"], np.float32)
    key = np.asarray(inputs["key"], np.float32)
    value = np.asarray(inputs["value"], np.float32)
    ipw = np.asarray(inputs["in_proj_weight"], np.float32)
    ipb = np.asarray(inputs["in_proj_bias"], np.float32)
    opw = np.asarray(inputs["out_proj_weight"], np.float32)
    opb = np.asarray(inputs["out_proj_bias"], np.float32)
    n1w = np.asarray(inputs["norm1_w"], np.float32)
    n1b = np.asarray(inputs["norm1_b"], np.float32)
    w1 = np.asarray(inputs["ffn_w1"], np.float32)
    b1 = np.asarray(inputs["ffn_b1"], np.float32)
    w2 = np.asarray(inputs["ffn_w2"], np.float32)
    b2 = np.asarray(inputs["ffn_b2"], np.float32)
    rf = np.asarray(inputs["ranks_feat_f"], np.int64)
    rb = np.asarray(inputs["ranks_bev_f"], np.int64)
    head_dim = int(np.asarray(inputs["head_dim"]))
    scaling = float(head_dim) ** -0.5

    # Segment structure straight from ranks_bev (sorted; constant per cell).
    lengths = np.bincount(rb, minlength=TGT).astype(np.int64)
    starts = np.concatenate([[0], np.cumsum(lengths)[:-1]])

    bin_of, slot_of = _pack_bins(lengths)
    core_of_bin = np.arange(NBINS) % NCORES
    local_bin = np.arange(NBINS) // NCORES

    bin_pts = np.zeros(NBINS, np.int64)
    np.add.at(bin_pts, bin_of, lengths)
    B = int(np.ceil(bin_pts.max() / 128))
    PTS = NB * B * 128          # point slots per core

    f_idx = np.zeros((NCORES, PTS), np.int16)
    b_loc = np.full((NCORES, PTS), -1.0, np.float32)
    query_core = np.zeros((NCORES, SLOTS_PAD, EMBED), np.float32)
    cell_of_slot = np.full((NCORES, SLOTS_PAD), -1, np.int64)

    fill = np.zeros(NBINS, np.int64)
    cell_order = np.lexsort((slot_of, bin_of))
    for cell in cell_order:
        g = bin_of[cell]
        c = core_of_bin[g]
        lb = local_bin[g]
        s = slot_of[cell]
        L = int(lengths[cell])
        gslot = lb * CPB + s
        cell_of_slot[c, gslot] = cell
        query_core[c, gslot] = q_full[cell]
        if L == 0:
            continue
        p0 = lb * B * 128 + fill[g]
        sl = slice(int(starts[cell]), int(starts[cell]) + L)
        f_idx[c, p0:p0 + L] = rf[sl].astype(np.int16)
        b_loc[c, p0:p0 + L] = s
        fill[g] += L

    # Gather index layout: within each bin's B*128 span, index j ->
    # [j % 16, col0 + j // 16], replicated across the 8 Q7 stripes.
    v = f_idx.reshape(NCORES, NB, B * 8, 16)
    f_wr = np.tile(
        v.transpose(0, 3, 1, 2).reshape(NCORES, 16, NB * B * 8), (1, 8, 1)
    )

    # Selection matrices, host-built in bf16 (exact 0/1):
    #   S   [128, NB*B*36]: point-major, for the segment-reduce matmul
    #   S^T [36, NB*B*128]: cell-major, for the q-expansion matmul
    bl3 = b_loc.reshape(NCORES, NB * B, 128)
    iot = np.arange(CPB, dtype=np.float32)
    S_pm = bl3[:, :, :, None] == iot[None, None, None, :]  # [C, NB*B, 128, 36]
    S_host = np.ascontiguousarray(
        S_pm.transpose(0, 2, 1, 3).reshape(NCORES, 128, NB * B * CPB)
    ).astype(NPBF)
    ST_host = np.ascontiguousarray(
        S_pm.transpose(0, 3, 1, 2).reshape(NCORES, CPB, NB * B * 128)
    ).astype(NPBF)

    Wk = ipw[:EMBED]
    Wq = ipw[2 * EMBED:3 * EMBED]
    shared = {
        "keyT": np.ascontiguousarray(key.T).astype(NPBF),         # [256, SRC]
        "WkT": np.ascontiguousarray(Wk.T).astype(NPBF),           # [256, 256]
        "WqTs": np.ascontiguousarray(Wq.T * scaling).astype(NPBF),
        "valueB": value.astype(NPBF),                             # [SRC, 256]
        "WoutT": np.ascontiguousarray(opw.T).astype(NPBF),        # [256, 256]
        "W1T": np.ascontiguousarray(w1.T).astype(NPBF),           # [256, 512]
        "W2T": np.ascontiguousarray(w2.T).astype(NPBF),           # [512, 256]
        "rowvecs": np.stack([ipb[:EMBED], ipb[2 * EMBED:] * scaling, n1w, n1b]),
        "bcol1": np.ascontiguousarray(b1.reshape(4, 128).T),      # [128, 4]
        "bcol2": np.ascontiguousarray(b2.reshape(2, 128).T),      # [128, 2]
    }

    in_maps = []
    for c in range(NCORES):
        m = dict(shared)
        m["f_wr"] = f_wr[c]
        m["S_in"] = S_host[c]
        m["ST_in"] = ST_host[c]
        qT = query_core[c].T + opb[:, None]       # fold out_proj bias
        m["queryT"] = np.ascontiguousarray(qT)                # f32 [256, 4096]
        m["queryTB"] = np.ascontiguousarray(qT).astype(NPBF)  # bf16 copy
        in_maps.append(m)

    return in_maps, cell_of_slot, B


_PROG_CACHE = {}


def _build_program(B):
    nc = bacc.Bacc("TRN2", target_bir_lowering=False, debug=False)

    keyT = nc.dram_tensor("keyT", [EMBED, SRC], BF16, kind="ExternalInput")
    WkT = nc.dram_tensor("WkT", [EMBED, EMBED], BF16, kind="ExternalInput")
    WqTs = nc.dram_tensor("WqTs", [EMBED, EMBED], BF16, kind="ExternalInput")
    valueB = nc.dram_tensor("valueB", [SRC, EMBED], BF16, kind="ExternalInput")
    WoutT = nc.dram_tensor("WoutT", [EMBED, EMBED], BF16, kind="ExternalInput")
    W1T = nc.dram_tensor("W1T", [EMBED, 2 * EMBED], BF16, kind="ExternalInput")
    W2T = nc.dram_tensor("W2T", [2 * EMBED, EMBED], BF16, kind="ExternalInput")
    rowvecs = nc.dram_tensor("rowvecs", [4, EMBED], F32, kind="ExternalInput")
    bcol1 = nc.dram_tensor("bcol1", [128, 4], F32, kind="ExternalInput")
    bcol2 = nc.dram_tensor("bcol2", [128, 2], F32, kind="ExternalInput")
    f_wr = nc.dram_tensor("f_wr", [128, NB * B * 8], I16, kind="ExternalInput")
    S_in = nc.dram_tensor("S_in", [128, NB * B * CPB], BF16, kind="ExternalInput")
    ST_in = nc.dram_tensor(
        "ST_in", [CPB, NB * B * 128], BF16, kind="ExternalInput"
    )
    queryT = nc.dram_tensor("queryT", [EMBED, SLOTS_PAD], F32, kind="ExternalInput")
    queryTB = nc.dram_tensor(
        "queryTB", [EMBED, SLOTS_PAD], BF16, kind="ExternalInput"
    )

    kv_cat = nc.dram_tensor("kv_cat", [SRC, 2 * EMBED], BF16, kind="Internal")
    qproj = nc.dram_tensor("qproj", [SLOTS_PAD, EMBED], BF16, kind="Internal")
    attn = nc.dram_tensor("attn", [SLOTS_PAD, EMBED], BF16, kind="Internal")
    outT = nc.dram_tensor("outT", [EMBED, SLOTS_PAD], F32, kind="ExternalOutput")

    with TileContext(nc) as tc:
        with tc.tile_pool(name="const", bufs=1) as cp:
            idxf_sb = cp.tile([128, NB * B * 8], I16)
            nc.sync.dma_start(out=idxf_sb[:], in_=f_wr[:, :])
            ident = cp.tile([128, 128], BF16)
            make_identity(nc, ident[:])
            ident32 = cp.tile([128, 128], F32)
            make_identity(nc, ident32[:])
            wk_sb = cp.tile([128, 2 * EMBED], BF16)
            nc.sync.dma_start(
                out=wk_sb[:].rearrange("p (c n) -> p c n", c=2),
                in_=WkT[:, :].rearrange("(c p) n -> p c n", p=128),
            )
            wq_sb = cp.tile([128, 2 * EMBED], BF16)
            nc.sync.dma_start(
                out=wq_sb[:].rearrange("p (c n) -> p c n", c=2),
                in_=WqTs[:, :].rearrange("(c p) n -> p c n", p=128),
            )
            wout_sb = cp.tile([128, 4 * 128], BF16)
            nc.sync.dma_start(
                out=wout_sb[:].rearrange("p (k m n) -> p k m n", k=2, m=2),
                in_=WoutT[:, :].rearrange("(k p) (m n) -> p k m n", p=128, n=128),
            )
            w1_sb = cp.tile([128, 8 * 128], BF16)
            nc.sync.dma_start(
                out=w1_sb[:].rearrange("p (k m n) -> p k m n", k=2, m=4),
                in_=W1T[:, :].rearrange("(k p) (m n) -> p k m n", p=128, n=128),
            )
            w2_sb = cp.tile([128, 8 * 128], BF16)
            nc.sync.dma_start(
                out=w2_sb[:].rearrange("p (k m n) -> p k m n", k=4, m=2),
                in_=W2T[:, :].rearrange("(k p) (m n) -> p k m n", p=128, n=128),
            )
            bc1_sb = cp.tile([128, 4], F32)
            nc.sync.dma_start(out=bc1_sb[:], in_=bcol1[:, :])
            bc2_sb = cp.tile([128, 2], F32)
            nc.sync.dma_start(out=bc2_sb[:], in_=bcol2[:, :])
            rv_stage = cp.tile([128, EMBED], F32)
            reps = []
            for k in range(4):
                rep = cp.tile([128, EMBED], F32, tag=f"rep{k}", name=f"rep{k}")
                nc.sync.dma_start(out=rv_stage[0:1, :], in_=rowvecs[k:k + 1, :])
                nc.gpsimd.partition_broadcast(rep[:], rv_stage[0:1, :])
                reps.append(rep)
            rep_bk, rep_bq, rep_nw, rep_nb = reps

            # ---- pass 0: projections into kv_cat / qproj ----
            with (
                tc.tile_pool(name="p0src", bufs=1) as p0src,
                tc.tile_pool(name="p0", bufs=3) as p0,
                tc.tile_pool(name="p0ps", bufs=3, space="PSUM") as p0ps,
            ):
                zt = p0.tile([SLOTS_PAD - SLOTS, EMBED], BF16, tag="zt")
                nc.vector.memset(zt[:], 0.0)
                nc.sync.dma_start(out=attn[SLOTS:SLOTS_PAD, :], in_=zt[:])
                # raw value half of the kv table
                nc.sync.dma_start(
                    out=kv_cat[:, EMBED:2 * EMBED], in_=valueB[:, :]
                )
                keyT_sb = p0src.tile([128, 2 * SRC], BF16)
                nc.sync.dma_start(
                    out=keyT_sb[:].rearrange("p (c n) -> p c n", c=2),
                    in_=keyT[:, :].rearrange("(c p) n -> p c n", p=128),
                )
                qTB_sb = p0src.tile([128, 2 * SLOTS_PAD], BF16)
                nc.sync.dma_start(
                    out=qTB_sb[:].rearrange("p (c n) -> p c n", c=2),
                    in_=queryTB[:, :].rearrange("(c p) n -> p c n", p=128),
                )

                def proj(dst4, src_sb, ncols, w_sb, rep_bias):
                    src_v = src_sb[:].rearrange("p (c n) -> p c n", c=2)
                    w_v = w_sb[:].rearrange("p (c n) -> p c n", c=2)
                    n4 = ncols // 512
                    for t4 in range(n4):
                        row4 = p0.tile([128, 4 * EMBED], BF16, tag="row",
                                       name="row")
                        for u in range(4):
                            t = t4 * 4 + u
                            ps = p0ps.tile([128, EMBED], F32, tag="ps",
                                           name="ps")
                            nc.tensor.matmul(
                                ps[:], src_v[:, 0, bass.ts(t, 128)],
                                w_v[:, 0, :], start=True, stop=False,
                            )
                            nc.tensor.matmul(
                                ps[:], src_v[:, 1, bass.ts(t, 128)],
                                w_v[:, 1, :], start=False, stop=True,
                            )
                            nc.vector.tensor_add(
                                row4[:, bass.ts(u, EMBED)], ps[:], rep_bias[:]
                            )
                        nc.sync.dma_start(out=dst4(t4), in_=row4[:])

                proj(
                    lambda t4: kv_cat[bass.ts(t4, 512), 0:EMBED]
                    .rearrange("(u p) n -> p u n", p=128),
                    keyT_sb, SRC, wk_sb, rep_bk,
                )
                proj(
                    lambda t4: qproj[bass.ts(t4, 512), :]
                    .rearrange("(u p) n -> p u n", p=128),
                    qTB_sb, SLOTS_PAD, wq_sb, rep_bq,
                )

            # ---- pass 1: gather attention per bin ----
            GB = 2                      # bins per gather
            with (
                tc.tile_pool(name="p1g", bufs=3) as p1g,
                tc.tile_pool(name="p1", bufs=2) as p1,
                tc.tile_pool(name="p1ps", bufs=2, space="PSUM") as p1ps,
                tc.tile_pool(name="p1qs", bufs=2, space="PSUM") as p1qs,
                tc.tile_pool(name="p2", bufs=2) as p2,
                tc.tile_pool(name="p2ps", bufs=2, space="PSUM") as p2ps,
            ):
                wout_v = wout_sb[:].rearrange("p (k m n) -> p k m n", k=2, m=2)
                w1_v = w1_sb[:].rearrange("p (k m n) -> p k m n", k=2, m=4)
                w2_v = w2_sb[:].rearrange("p (k m n) -> p k m n", k=4, m=2)

                def transpose4(dst_list, src_of, dt, idn):
                    for cch in range(2):
                        for t in range(4):
                            tp = p2ps.tile([128, 512], dt, tag="ps2",
                                           name=f"tp{cch}_{t}")
                            nc.tensor.matmul(
                                tp[:, 0:128], src_of(t, cch), idn[:],
                                start=True, stop=True, is_transpose=True,
                            )
                            nc.vector.tensor_copy(
                                dst_list[cch][:, bass.ts(t, 128)], tp[:, 0:128]
                            )

                def emit_pass2(it):
                    A4 = p2.tile([128, 4 * EMBED], BF16, tag="A4", name="A4")
                    nc.sync.dma_start(
                        out=A4[:].rearrange("p (t n) -> p t n", t=4),
                        in_=attn[bass.ts(it, 512), :]
                        .rearrange("(t p) n -> p t n", p=128),
                    )
                    A4v = A4[:].rearrange("p (t n) -> p t n", t=4)
                    AT4 = [p2.tile([128, 512], BF16, tag=f"AT{i}", name=f"AT{i}")
                           for i in range(2)]
                    transpose4(
                        AT4,
                        lambda t, cc: A4v[:, t, bass.ts(cc, 128)],
                        BF16, ident,
                    )
                    zT4 = [p2.tile([128, 512], F32, tag=f"zT{i}", name=f"zT{i}")
                           for i in range(2)]
                    for mch in range(2):
                        yp = p2ps.tile([128, 512], F32, tag="ps2", name="yp")
                        for kch in range(2):
                            nc.tensor.matmul(
                                yp[:], wout_v[:, kch, mch, :], AT4[kch][:],
                                start=(kch == 0), stop=(kch == 1),
                            )
                        qt = p2.tile([128, 512], F32, tag="qt", name="qt")
                        nc.sync.dma_start(
                            out=qt[:],
                            in_=queryT[bass.ts(mch, 128), bass.ts(it, 512)],
                        )
                        nc.vector.tensor_add(zT4[mch][:], yp[:], qt[:])
                    z4 = p2.tile([128, 4 * EMBED], F32, tag="z4", name="z4")
                    z4v = z4[:].rearrange("p (t n) -> p t n", t=4)
                    for cch in range(2):
                        for t in range(4):
                            tp2 = p2ps.tile([128, 512], F32, tag="ps2",
                                            name="tp2")
                            nc.tensor.matmul(
                                tp2[:, 0:128], zT4[cch][:, bass.ts(t, 128)],
                                ident32[:], start=True, stop=True,
                                is_transpose=True,
                            )
                            nc.vector.tensor_copy(
                                z4v[:, t, bass.ts(cch, 128)], tp2[:, 0:128]
                            )
                    mu = p2.tile([128, 4], F32, tag="mu", name="mu")
                    nc.vector.reduce_sum(mu[:], z4v, axis=mybir.AxisListType.X)
                    nc.vector.tensor_scalar_mul(mu[:], mu[:], 1.0 / EMBED)
                    zc = p2.tile([128, 4 * EMBED], F32, tag="zc", name="zc")
                    zcv = zc[:].rearrange("p (t n) -> p t n", t=4)
                    nc.vector.tensor_sub(
                        zcv, z4v, mu[:][:, :, None].to_broadcast([128, 4, EMBED])
                    )
                    sq = p2.tile([128, 4 * EMBED], F32, tag="sq", name="sq")
                    nc.scalar.square(sq[:], zc[:])
                    var = p2.tile([128, 4], F32, tag="var", name="var")
                    nc.vector.reduce_sum(
                        var[:], sq[:].rearrange("p (t n) -> p t n", t=4),
                        axis=mybir.AxisListType.X,
                    )
                    nc.vector.tensor_scalar_mul(var[:], var[:], 1.0 / EMBED)
                    nc.vector.tensor_scalar_add(var[:], var[:], 1e-5)
                    sd = p2.tile([128, 4], F32, tag="sd", name="sd")
                    nc.scalar.sqrt(sd[:], var[:])
                    rstd = p2.tile([128, 4], F32, tag="rstd", name="rstd")
                    nc.vector.reciprocal(rstd[:], sd[:])
                    xh = p2.tile([128, 4 * EMBED], F32, tag="xh", name="xh")
                    xhv = xh[:].rearrange("p (t n) -> p t n", t=4)
                    nc.vector.tensor_mul(
                        xhv, zcv,
                        rstd[:][:, :, None].to_broadcast([128, 4, EMBED]),
                    )
                    nc.vector.tensor_mul(
                        xhv, xhv,
                        rep_nw[:][:, None, :].to_broadcast([128, 4, EMBED]),
                    )
                    xhb = p2.tile([128, 4 * EMBED], BF16, tag="xhb", name="xhb")
                    xhbv = xhb[:].rearrange("p (t n) -> p t n", t=4)
                    nc.vector.tensor_add(
                        xhbv, xhv,
                        rep_nb[:][:, None, :].to_broadcast([128, 4, EMBED]),
                    )
                    xT4 = [p2.tile([128, 512], BF16, tag=f"xT{i}", name=f"xT{i}")
                           for i in range(2)]
                    transpose4(
                        xT4,
                        lambda t, cc: xhbv[:, t, bass.ts(cc, 128)],
                        BF16, ident,
                    )
                    h4 = [p2.tile([128, 512], BF16, tag=f"h{i}", name=f"h{i}")
                          for i in range(4)]
                    for mch in range(4):
                        hp = p2ps.tile([128, 512], F32, tag="ps2", name="hp")
                        for kch in range(2):
                            nc.tensor.matmul(
                                hp[:], w1_v[:, kch, mch, :], xT4[kch][:],
                                start=(kch == 0), stop=(kch == 1),
                            )
                        nc.scalar.activation(
                            h4[mch][:], hp[:], mybir.ActivationFunctionType.Relu,
                            bias=bc1_sb[:, mch:mch + 1],
                        )
                    for mch in range(2):
                        op = p2ps.tile([128, 512], F32, tag="ps2", name="op")
                        for kch in range(4):
                            nc.tensor.matmul(
                                op[:], w2_v[:, kch, mch, :], h4[kch][:],
                                start=(kch == 0), stop=(kch == 3),
                            )
                        o1 = p2.tile([128, 512], F32, tag="o1", name="o1")
                        nc.scalar.activation(
                            o1[:], op[:], mybir.ActivationFunctionType.Identity,
                            bias=bc2_sb[:, mch:mch + 1],
                        )
                        nc.vector.tensor_add(o1[:], o1[:], xT4[mch][:])
                        nc.sync.dma_start(
                            out=outT[bass.ts(mch, 128), bass.ts(it, 512)],
                            in_=o1[:],
                        )

                # bin after which pass-2 iteration `it` becomes ready
                p2_after = {}
                for it in range(NT2 // 4):
                    need = min(NB, -(-((it + 1) * 512) // CPB))
                    p2_after.setdefault(need - 1, []).append(it)

                kvg = None
                for lb in range(NB):
                    if lb % GB == 0:
                        nbin = min(GB, NB - lb)
                        nidx = nbin * B * 128
                        ic0 = lb * B * 8
                        kvg = p1g.tile(
                            [128, GB * B * 2 * EMBED], BF16, tag="kvg",
                            name=f"kvg{lb}",
                        )
                        nc.gpsimd.dma_gather(
                            kvg[:].rearrange(
                                "p (b n) -> p b n", n=2 * EMBED
                            )[:, 0:nbin * B, :],
                            kv_cat[:, :],
                            idxf_sb[:, ic0:ic0 + nbin * B * 8],
                            num_idxs=nidx, num_idxs_reg=nidx,
                            elem_size=2 * EMBED, single_packet=False,
                        )
                    kvv = kvg[:].rearrange("p (b n) -> p b n", n=2 * EMBED)
                    boff = (lb % GB) * B

                    st_sb = p1.tile([CPB, B * 128], BF16, tag="st", name="st")
                    nc.sync.dma_start(
                        out=st_sb[:],
                        in_=ST_in[:, lb * B * 128:(lb + 1) * B * 128],
                    )
                    s_sb = p1.tile([128, B * CPB], BF16, tag="s", name="s")
                    nc.scalar.dma_start(
                        out=s_sb[:], in_=S_in[:, lb * B * CPB:(lb + 1) * B * CPB]
                    )
                    qc_sb = p1.tile([CPB, EMBED], BF16, tag="qc", name="qc")
                    nc.scalar.dma_start(
                        out=qc_sb[:], in_=qproj[lb * CPB:(lb + 1) * CPB, :]
                    )

                    ebin = p1.tile([128, B * HEADS], F32, tag="ebin", name="ebin")
                    for j0 in range(0, B, 3):
                        g = min(3, B - j0)
                        qg_ps = p1qs.tile(
                            [128, g * EMBED], F32, tag="qg", name=f"qg{lb}_{j0}"
                        )
                        for j in range(j0, j0 + g):
                            nc.tensor.matmul(
                                qg_ps[:, bass.ts(j - j0, EMBED)],
                                st_sb[:, bass.ts(j, 128)], qc_sb[:],
                                start=True, stop=True,
                            )
                        prod = p1.tile(
                            [128, g * EMBED], BF16, tag="prod",
                            name=f"prod{lb}_{j0}",
                        )
                        nc.vector.tensor_mul(
                            prod[:].rearrange("p (b n) -> p b n", n=EMBED),
                            kvv[:, boff + j0:boff + j0 + g, 0:EMBED],
                            qg_ps[:].rearrange("p (b n) -> p b n", n=EMBED),
                        )
                        nc.vector.reduce_sum(
                            ebin[:, j0 * HEADS:(j0 + g) * HEADS]
                            .rearrange("p (o h) -> p o h", o=1),
                            prod[:].rearrange("p (h d) -> p h d", d=HD),
                            axis=mybir.AxisListType.X,
                        )
                    EXT = EMBED + HEADS
                    pvb = p1.tile([128, B * EXT], BF16, tag="pv", name=f"pv{lb}")
                    pvbv = pvb[:].rearrange("p (b n) -> p b n", n=EXT)
                    nc.scalar.activation(
                        pvbv[:, :, EMBED:EXT],
                        ebin[:].rearrange("p (b h) -> p b h", h=HEADS),
                        mybir.ActivationFunctionType.Exp,
                    )
                    oc_ps = p1ps.tile([CPB, EXT], F32, tag="oc", name="oc")
                    nc.vector.tensor_mul(
                        pvbv[:, :, 0:EMBED]
                        .rearrange("p b (h d) -> p b h d", d=HD),
                        kvv[:, boff:boff + B, EMBED:2 * EMBED]
                        .rearrange("p b (h d) -> p b h d", d=HD),
                        pvbv[:, :, EMBED:EXT][:, :, :, None]
                        .to_broadcast([128, B, HEADS, HD]),
                    )
                    for j in range(B):
                        nc.tensor.matmul(
                            oc_ps[:], s_sb[:, bass.ts(j, CPB)],
                            pvb[:, bass.ts(j, EXT)],
                            start=(j == 0), stop=(j == B - 1),
                        )
                    dn = p1.tile([CPB, HEADS], F32, tag="dnsb", name="dnsb")
                    nc.vector.tensor_scalar_add(
                        dn[:], oc_ps[:, EMBED:EXT], 1e-30
                    )
                    rcp = p1.tile([CPB, HEADS], F32, tag="rcp", name="rcp")
                    nc.vector.reciprocal(rcp[:], dn[:])
                    an = p1.tile([CPB, EMBED], BF16, tag="an", name="an")
                    nc.vector.tensor_mul(
                        an[:].rearrange("p (h d) -> p h d", d=HD),
                        oc_ps[:, 0:EMBED].rearrange("p (h d) -> p h d", d=HD),
                        rcp[:][:, :, None].to_broadcast([CPB, HEADS, HD]),
                    )
                    nc.sync.dma_start(
                        out=attn[lb * CPB:(lb + 1) * CPB, :], in_=an[:]
                    )
                    for it in p2_after.get(lb, []):
                        emit_pass2(it)

            # ---- pass 2: (interleaved above) ----
    nc.compile()
    return nc


def kernel(**inputs):
    in_maps, cell_of_slot, B = _host_prep(inputs)
    if B not in _PROG_CACHE:
        _PROG_CACHE[B] = _build_program(B)
    nc = _PROG_CACHE[B]
    res = bass_utils.run_bass_kernel_spmd(nc, in_maps, core_ids=list(range(NCORES)))
    out = np.zeros((TGT, EMBED), np.float32)
    for c in range(NCORES):
        oc = res.results[c]["outT"].T  # [4096, 256]
        mask = cell_of_slot[c] >= 0
        out[cell_of_slot[c][mask]] = oc[mask]
    return out


# revision 21
# speedup vs baseline: 1.0008x; 1.0008x over previous
"""DepthAttnLayer Trainium2 kernel: ragged gather-attention over BEV cells.

Strategy (SPMD over 8 cores, one shared program):
  * Host repacks the 32400 ragged BEV cells into 904 uniform "bins" of
    exactly <=36 cells (LPT-balanced so every bin is <= B*128 points),
    113 bins per core; every bin's points padded to B*128 point-slots so
    the device program is identical across cores and bins.
  * Pass 0: k/q in-projections on the PE (bf16); projected-k rows and raw
    value rows are packed side by side into one [SRC, 512] bf16 table so a
    single 1KB-row dma_gather fetches both per point (descriptor count is
    the bottleneck: the Q7 SWDGE generates ~8ns/descriptor).
  * Pass 1 (per bin): dma_gather of kv rows; per-point q is expanded from
    the bin's 36 query rows by a PE matmul with a host-shipped 0/1
    selection matrix S^T (no q gather); per-point q*k head-dot on DVE;
    interval softmax via exp (logits are small, no max-subtract) with the
    per-cell 1/denom applied after the segment reduce; segment reduce back
    to cells with S matmuls on the PE.
  * Pass 2: out-proj + residual + LayerNorm + FFN in 128-row tiles,
    transposing between row-major (LN) and feature-major (matmuls) on PE.
"""
import os
import sys

for _p in ("/opt/trn_rl_repo", "/root/.axon_site/_ro/trn_rl_repo"):
    if os.path.isdir(_p) and _p not in sys.path:
        sys.path.insert(0, _p)

import heapq

import ml_dtypes
import numpy as np

import concourse.bacc as bacc
import concourse.bass as bass
import concourse.mybir as mybir
from concourse import bass_utils
from concourse.masks import make_identity
from concourse.tile import TileContext

F32 = mybir.dt.float32
BF16 = mybir.dt.bfloat16
I16 = mybir.dt.int16
NPBF = ml_dtypes.bfloat16

EMBED = 256
HEADS = 8
HD = 32
TGT = 32400
SRC = 16896
NCORES = 8
CPB = 36                      # cell slots per bin
NBINS = 904                   # total bins (multiple of NCORES)
NB = NBINS // NCORES          # bins per core = 113
SLOTS = NB * CPB              # cell slots per core = 4068
SLOTS_PAD = 4096              # attn/out rows per core (32 tiles of 128)
NT2 = SLOTS_PAD // 128        # pass-2 tiles


def _pack_bins(lengths):
    """LPT-pack cells into NBINS bins of exactly <=CPB slots.

    Returns (bin_of_cell, slot_of_cell)."""
    order = np.argsort(-lengths, kind="stable")
    bin_of = np.empty(TGT, np.int32)
    slot_of = np.empty(TGT, np.int32)
    used = np.zeros(NBINS, np.int32)
    pts = np.zeros(NBINS, np.int64)
    heap = [(0, b) for b in range(NBINS)]
    heapq.heapify(heap)
    for cell in order:
        while True:
            p, b = heapq.heappop(heap)
            if used[b] < CPB and p == pts[b]:
                break
        bin_of[cell] = b
        slot_of[cell] = used[b]
        used[b] += 1
        pts[b] += lengths[cell]
        if used[b] < CPB:
            heapq.heappush(heap, (int(pts[b]), b))
    return bin_of, slot_of


def _host_prep(inputs):
    q_full = np.asarray(inputs["query_depth"], np.float32)
    key = np.asarray(inputs["key"], np.float32)
    value = np.asarray(inputs["value"], np.float32)
    ipw = np.asarray(inputs["in_proj_weight"], np.float32)
    ipb = np.asarray(inputs["in_proj_bias"], np.float32)
    opw = np.asarray(inputs["out_proj_weight"], np.float32)
    opb = np.asarray(inputs["out_proj_bias"], np.float32)
    n1w = np.asarray(inputs["norm1_w"], np.float32)
    n1b = np.asarray(inputs["norm1_b"], np.float32)
    w1 = np.asarray(inputs["ffn_w1"], np.float32)
    b1 = np.asarray(inputs["ffn_b1"], np.float32)
    w2 = np.asarray(inputs["ffn_w2"], np.float32)
    b2 = np.asarray(inputs["ffn_b2"], np.float32)
    rf = np.asarray(inputs["ranks_feat_f"], np.int64)
    rb = np.asarray(inputs["ranks_bev_f"], np.int64)
    head_dim = int(np.asarray(inputs["head_dim"]))
    scaling = float(head_dim) ** -0.5

    # Segment structure straight from ranks_bev (sorted; constant per cell).
    lengths = np.bincount(rb, minlength=TGT).astype(np.int64)
    starts = np.concatenate([[0], np.cumsum(lengths)[:-1]])

    bin_of, slot_of = _pack_bins(lengths)
    core_of_bin = np.arange(NBINS) % NCORES
    local_bin = np.arange(NBINS) // NCORES

    bin_pts = np.zeros(NBINS, np.int64)
    np.add.at(bin_pts, bin_of, lengths)
    B = int(np.ceil(bin_pts.max() / 128))
    PTS = NB * B * 128          # point slots per core

    f_idx = np.zeros((NCORES, PTS), np.int16)
    b_loc = np.full((NCORES, PTS), -1.0, np.float32)
    query_core = np.zeros((NCORES, SLOTS_PAD, EMBED), np.float32)
    cell_of_slot = np.full((NCORES, SLOTS_PAD), -1, np.int64)

    fill = np.zeros(NBINS, np.int64)
    cell_order = np.lexsort((slot_of, bin_of))
    for cell in cell_order:
        g = bin_of[cell]
        c = core_of_bin[g]
        lb = local_bin[g]
        s = slot_of[cell]
        L = int(lengths[cell])
        gslot = lb * CPB + s
        cell_of_slot[c, gslot] = cell
        query_core[c, gslot] = q_full[cell]
        if L == 0:
            continue
        p0 = lb * B * 128 + fill[g]
        sl = slice(int(starts[cell]), int(starts[cell]) + L)
        f_idx[c, p0:p0 + L] = rf[sl].astype(np.int16)
        b_loc[c, p0:p0 + L] = s
        fill[g] += L

    # Gather index layout: within each bin's B*128 span, index j ->
    # [j % 16, col0 + j // 16], replicated across the 8 Q7 stripes.
    v = f_idx.reshape(NCORES, NB, B * 8, 16)
    f_wr = np.tile(
        v.transpose(0, 3, 1, 2).reshape(NCORES, 16, NB * B * 8), (1, 8, 1)
    )

    # Selection matrices, host-built in bf16 (exact 0/1):
    #   S   [128, NB*B*36]: point-major, for the segment-reduce matmul
    #   S^T [36, NB*B*128]: cell-major, for the q-expansion matmul
    bl3 = b_loc.reshape(NCORES, NB * B, 128)
    iot = np.arange(CPB, dtype=np.float32)
    S_pm = bl3[:, :, :, None] == iot[None, None, None, :]  # [C, NB*B, 128, 36]
    S_host = np.ascontiguousarray(
        S_pm.transpose(0, 2, 1, 3).reshape(NCORES, 128, NB * B * CPB)
    ).astype(NPBF)
    ST_host = np.ascontiguousarray(
        S_pm.transpose(0, 3, 1, 2).reshape(NCORES, CPB, NB * B * 128)
    ).astype(NPBF)

    Wk = ipw[:EMBED]
    Wq = ipw[2 * EMBED:3 * EMBED]
    shared = {
        "keyT": np.ascontiguousarray(key.T).astype(NPBF),         # [256, SRC]
        "WkT": np.ascontiguousarray(Wk.T).astype(NPBF),           # [256, 256]
        "WqTs": np.ascontiguousarray(Wq.T * scaling).astype(NPBF),
        "valueB": value.astype(NPBF),                             # [SRC, 256]
        "WoutT": np.ascontiguousarray(opw.T).astype(NPBF),        # [256, 256]
        "W1T": np.ascontiguousarray(w1.T).astype(NPBF),           # [256, 512]
        "W2T": np.ascontiguousarray(w2.T).astype(NPBF),           # [512, 256]
        "rowvecs": np.stack([ipb[:EMBED], ipb[2 * EMBED:] * scaling, n1w, n1b]),
        "bcol1": np.ascontiguousarray(b1.reshape(4, 128).T),      # [128, 4]
        "bcol2": np.ascontiguousarray(b2.reshape(2, 128).T),      # [128, 2]
    }

    in_maps = []
    for c in range(NCORES):
        m = dict(shared)
        m["f_wr"] = f_wr[c]
        m["S_in"] = S_host[c]
        m["ST_in"] = ST_host[c]
        qT = query_core[c].T + opb[:, None]       # fold out_proj bias
        m["queryT"] = np.ascontiguousarray(qT)                # f32 [256, 4096]
        m["queryTB"] = np.ascontiguousarray(qT).astype(NPBF)  # bf16 copy
        in_maps.append(m)

    return in_maps, cell_of_slot, B


_PROG_CACHE = {}


def _build_program(B):
    nc = bacc.Bacc("TRN2", target_bir_lowering=False, debug=False)

    keyT = nc.dram_tensor("keyT", [EMBED, SRC], BF16, kind="ExternalInput")
    WkT = nc.dram_tensor("WkT", [EMBED, EMBED], BF16, kind="ExternalInput")
    WqTs = nc.dram_tensor("WqTs", [EMBED, EMBED], BF16, kind="ExternalInput")
    valueB = nc.dram_tensor("valueB", [SRC, EMBED], BF16, kind="ExternalInput")
    WoutT = nc.dram_tensor("WoutT", [EMBED, EMBED], BF16, kind="ExternalInput")
    W1T = nc.dram_tensor("W1T", [EMBED, 2 * EMBED], BF16, kind="ExternalInput")
    W2T = nc.dram_tensor("W2T", [2 * EMBED, EMBED], BF16, kind="ExternalInput")
    rowvecs = nc.dram_tensor("rowvecs", [4, EMBED], F32, kind="ExternalInput")
    bcol1 = nc.dram_tensor("bcol1", [128, 4], F32, kind="ExternalInput")
    bcol2 = nc.dram_tensor("bcol2", [128, 2], F32, kind="ExternalInput")
    f_wr = nc.dram_tensor("f_wr", [128, NB * B * 8], I16, kind="ExternalInput")
    S_in = nc.dram_tensor("S_in", [128, NB * B * CPB], BF16, kind="ExternalInput")
    ST_in = nc.dram_tensor(
        "ST_in", [CPB, NB * B * 128], BF16, kind="ExternalInput"
    )
    queryT = nc.dram_tensor("queryT", [EMBED, SLOTS_PAD], F32, kind="ExternalInput")
    queryTB = nc.dram_tensor(
        "queryTB", [EMBED, SLOTS_PAD], BF16, kind="ExternalInput"
    )

    kv_cat = nc.dram_tensor("kv_cat", [SRC, 2 * EMBED], BF16, kind="Internal")
    qproj = nc.dram_tensor("qproj", [SLOTS_PAD, EMBED], BF16, kind="Internal")
    attn = nc.dram_tensor("attn", [SLOTS_PAD, EMBED], BF16, kind="Internal")
    outT = nc.dram_tensor("outT", [EMBED, SLOTS_PAD], F32, kind="ExternalOutput")

    with TileContext(nc) as tc:
        with tc.tile_pool(name="const", bufs=1) as cp:
            idxf_sb = cp.tile([128, NB * B * 8], I16)
            nc.sync.dma_start(out=idxf_sb[:], in_=f_wr[:, :])
            ident = cp.tile([128, 128], BF16)
            make_identity(nc, ident[:])
            ident32 = cp.tile([128, 128], F32)
            make_identity(nc, ident32[:])
            wk_sb = cp.tile([128, 2 * EMBED], BF16)
            nc.sync.dma_start(
                out=wk_sb[:].rearrange("p (c n) -> p c n", c=2),
                in_=WkT[:, :].rearrange("(c p) n -> p c n", p=128),
            )
            wq_sb = cp.tile([128, 2 * EMBED], BF16)
            nc.sync.dma_start(
                out=wq_sb[:].rearrange("p (c n) -> p c n", c=2),
                in_=WqTs[:, :].rearrange("(c p) n -> p c n", p=128),
            )
            wout_sb = cp.tile([128, 4 * 128], BF16)
            nc.sync.dma_start(
                out=wout_sb[:].rearrange("p (k m n) -> p k m n", k=2, m=2),
                in_=WoutT[:, :].rearrange("(k p) (m n) -> p k m n", p=128, n=128),
            )
            w1_sb = cp.tile([128, 8 * 128], BF16)
            nc.sync.dma_start(
                out=w1_sb[:].rearrange("p (k m n) -> p k m n", k=2, m=4),
                in_=W1T[:, :].rearrange("(k p) (m n) -> p k m n", p=128, n=128),
            )
            w2_sb = cp.tile([128, 8 * 128], BF16)
            nc.sync.dma_start(
                out=w2_sb[:].rearrange("p (k m n) -> p k m n", k=4, m=2),
                in_=W2T[:, :].rearrange("(k p) (m n) -> p k m n", p=128, n=128),
            )
            bc1_sb = cp.tile([128, 4], F32)
            nc.sync.dma_start(out=bc1_sb[:], in_=bcol1[:, :])
            bc2_sb = cp.tile([128, 2], F32)
            nc.sync.dma_start(out=bc2_sb[:], in_=bcol2[:, :])
            rv_stage = cp.tile([128, EMBED], F32)
            reps = []
            for k in range(4):
                rep = cp.tile([128, EMBED], F32, tag=f"rep{k}", name=f"rep{k}")
                nc.sync.dma_start(out=rv_stage[0:1, :], in_=rowvecs[k:k + 1, :])
                nc.gpsimd.partition_broadcast(rep[:], rv_stage[0:1, :])
                reps.append(rep)
            rep_bk, rep_bq, rep_nw, rep_nb = reps

            # ---- pass 0: projections into kv_cat / qproj ----
            with (
                tc.tile_pool(name="p0src", bufs=1) as p0src,
                tc.tile_pool(name="p0", bufs=3) as p0,
                tc.tile_pool(name="p0ps", bufs=3, space="PSUM") as p0ps,
            ):
                zt = p0.tile([SLOTS_PAD - SLOTS, EMBED], BF16, tag="zt")
                nc.vector.memset(zt[:], 0.0)
                nc.sync.dma_start(out=attn[SLOTS:SLOTS_PAD, :], in_=zt[:])
                # raw value half of the kv table
                nc.sync.dma_start(
                    out=kv_cat[:, EMBED:2 * EMBED], in_=valueB[:, :]
                )
                keyT_sb = p0src.tile([128, 2 * SRC], BF16)
                nc.sync.dma_start(
                    out=keyT_sb[:].rearrange("p (c n) -> p c n", c=2),
                    in_=keyT[:, :].rearrange("(c p) n -> p c n", p=128),
                )
                qTB_sb = p0src.tile([128, 2 * SLOTS_PAD], BF16)
                nc.sync.dma_start(
                    out=qTB_sb[:].rearrange("p (c n) -> p c n", c=2),
                    in_=queryTB[:, :].rearrange("(c p) n -> p c n", p=128),
                )

                def proj(dst4, src_sb, ncols, w_sb, rep_bias):
                    src_v = src_sb[:].rearrange("p (c n) -> p c n", c=2)
                    w_v = w_sb[:].rearrange("p (c n) -> p c n", c=2)
                    n4 = ncols // 512
                    for t4 in range(n4):
                        row4 = p0.tile([128, 4 * EMBED], BF16, tag="row",
                                       name="row")
                        for u in range(4):
                            t = t4 * 4 + u
                            ps = p0ps.tile([128, EMBED], F32, tag="ps",
                                           name="ps")
                            nc.tensor.matmul(
                                ps[:], src_v[:, 0, bass.ts(t, 128)],
                                w_v[:, 0, :], start=True, stop=False,
                            )
                            nc.tensor.matmul(
                                ps[:], src_v[:, 1, bass.ts(t, 128)],
                                w_v[:, 1, :], start=False, stop=True,
                            )
                            nc.vector.tensor_add(
                                row4[:, bass.ts(u, EMBED)], ps[:], rep_bias[:]
                            )
                        nc.sync.dma_start(out=dst4(t4), in_=row4[:])

                proj(
                    lambda t4: kv_cat[bass.ts(t4, 512), 0:EMBED]
                    .rearrange("(u p) n -> p u n", p=128),
                    keyT_sb, SRC, wk_sb, rep_bk,
                )
                proj(
                    lambda t4: qproj[bass.ts(t4, 512), :]
                    .rearrange("(u p) n -> p u n", p=128),
                    qTB_sb, SLOTS_PAD, wq_sb, rep_bq,
                )

            # ---- pass 1: gather attention per bin ----
            GB = 2                      # bins per gather
            with (
                tc.tile_pool(name="p1g", bufs=3) as p1g,
                tc.tile_pool(name="p1", bufs=2) as p1,
                tc.tile_pool(name="p1ps", bufs=2, space="PSUM") as p1ps,
                tc.tile_pool(name="p1qs", bufs=2, space="PSUM") as p1qs,
                tc.tile_pool(name="p2", bufs=2) as p2,
                tc.tile_pool(name="p2ps", bufs=2, space="PSUM") as p2ps,
            ):
                wout_v = wout_sb[:].rearrange("p (k m n) -> p k m n", k=2, m=2)
                w1_v = w1_sb[:].rearrange("p (k m n) -> p k m n", k=2, m=4)
                w2_v = w2_sb[:].rearrange("p (k m n) -> p k m n", k=4, m=2)

                def transpose4(dst_list, src_of, dt, idn):
                    for cch in range(2):
                        for t in range(4):
                            tp = p2ps.tile([128, 512], dt, tag="ps2",
                                           name=f"tp{cch}_{t}")
                            nc.tensor.matmul(
                                tp[:, 0:128], src_of(t, cch), idn[:],
                                start=True, stop=True, is_transpose=True,
                            )
                            nc.vector.tensor_copy(
                                dst_list[cch][:, bass.ts(t, 128)], tp[:, 0:128]
                            )

                def emit_pass2(it):
                    A4 = p2.tile([128, 4 * EMBED], BF16, tag="A4", name="A4")
                    nc.sync.dma_start(
                        out=A4[:].rearrange("p (t n) -> p t n", t=4),
                        in_=attn[bass.ts(it, 512), :]
                        .rearrange("(t p) n -> p t n", p=128),
                    )
                    A4v = A4[:].rearrange("p (t n) -> p t n", t=4)
                    AT4 = [p2.tile([128, 512], BF16, tag=f"AT{i}", name=f"AT{i}")
                           for i in range(2)]
                    transpose4(
                        AT4,
                        lambda t, cc: A4v[:, t, bass.ts(cc, 128)],
                        BF16, ident,
                    )
                    zT4 = [p2.tile([128, 512], F32, tag=f"zT{i}", name=f"zT{i}")
                           for i in range(2)]
                    for mch in range(2):
                        yp = p2ps.tile([128, 512], F32, tag="ps2", name="yp")
                        for kch in range(2):
                            nc.tensor.matmul(
                                yp[:], wout_v[:, kch, mch, :], AT4[kch][:],
                                start=(kch == 0), stop=(kch == 1),
                            )
                        qt = p2.tile([128, 512], F32, tag="qt", name="qt")
                        nc.sync.dma_start(
                            out=qt[:],
                            in_=queryT[bass.ts(mch, 128), bass.ts(it, 512)],
                        )
                        nc.vector.tensor_add(zT4[mch][:], yp[:], qt[:])
                    z4 = p2.tile([128, 4 * EMBED], F32, tag="z4", name="z4")
                    z4v = z4[:].rearrange("p (t n) -> p t n", t=4)
                    for cch in range(2):
                        for t in range(4):
                            tp2 = p2ps.tile([128, 512], F32, tag="ps2",
                                            name="tp2")
                            nc.tensor.matmul(
                                tp2[:, 0:128], zT4[cch][:, bass.ts(t, 128)],
                                ident32[:], start=True, stop=True,
                                is_transpose=True,
                            )
                            nc.vector.tensor_copy(
                                z4v[:, t, bass.ts(cch, 128)], tp2[:, 0:128]
                            )
                    mu = p2.tile([128, 4], F32, tag="mu", name="mu")
                    nc.vector.reduce_sum(mu[:], z4v, axis=mybir.AxisListType.X)
                    nc.vector.tensor_scalar_mul(mu[:], mu[:], 1.0 / EMBED)
                    zc = p2.tile([128, 4 * EMBED], F32, tag="zc", name="zc")
                    zcv = zc[:].rearrange("p (t n) -> p t n", t=4)
                    nc.vector.tensor_sub(
                        zcv, z4v, mu[:][:, :, None].to_broadcast([128, 4, EMBED])
                    )
                    sq = p2.tile([128, 4 * EMBED], F32, tag="sq", name="sq")
                    nc.scalar.square(sq[:], zc[:])
                    var = p2.tile([128, 4], F32, tag="var", name="var")
                    nc.vector.reduce_sum(
                        var[:], sq[:].rearrange("p (t n) -> p t n", t=4),
                        axis=mybir.AxisListType.X,
                    )
                    nc.vector.tensor_scalar_mul(var[:], var[:], 1.0 / EMBED)
                    nc.vector.tensor_scalar_add(var[:], var[:], 1e-5)
                    sd = p2.tile([128, 4], F32, tag="sd", name="sd")
                    nc.scalar.sqrt(sd[:], var[:])
                    rstd = p2.tile([128, 4], F32, tag="rstd", name="rstd")
                    nc.vector.reciprocal(rstd[:], sd[:])
                    xh = p2.tile([128, 4 * EMBED], F32, tag="xh", name="xh")
                    xhv = xh[:].rearrange("p (t n) -> p t n", t=4)
                    nc.vector.tensor_mul(
                        xhv, zcv,
                        rstd[:][:, :, None].to_broadcast([128, 4, EMBED]),
                    )
                    nc.vector.tensor_mul(
                        xhv, xhv,
                        rep_nw[:][:, None, :].to_broadcast([128, 4, EMBED]),
                    )
                    xhb = p2.tile([128, 4 * EMBED], BF16, tag="xhb", name="xhb")
                    xhbv = xhb[:].rearrange("p (t n) -> p t n", t=4)
                    nc.vector.tensor_add(
                        xhbv, xhv,
                        rep_nb[:][:, None, :].to_broadcast([128, 4, EMBED]),
                    )
                    xT4 = [p2.tile([128, 512], BF16, tag=f"xT{i}", name=f"xT{i}")
                           for i in range(2)]
                    transpose4(
                        xT4,
                        lambda t, cc: xhbv[:, t, bass.ts(cc, 128)],
                        BF16, ident,
                    )
                    h4 = [p2.tile([128, 512], BF16, tag=f"h{i}", name=f"h{i}")
                          for i in range(4)]
                    for mch in range(4):
                        hp = p2ps.tile([128, 512], F32, tag="ps2", name="hp")
                        for kch in range(2):
                            nc.tensor.matmul(
                                hp[:], w1_v[:, kch, mch, :], xT4[kch][:],
                                start=(kch == 0), stop=(kch == 1),
                            )
                        nc.scalar.activation(
                            h4[mch][:], hp[:], mybir.ActivationFunctionType.Relu,
                            bias=bc1_sb[:, mch:mch + 1],
                        )
                    for mch in range(2):
                        op = p2ps.tile([128, 512], F32, tag="ps2", name="op")
                        for kch in range(4):
                            nc.tensor.matmul(
                                op[:], w2_v[:, kch, mch, :], h4[kch][:],
                                start=(kch == 0), stop=(kch == 3),
                            )
                        o1 = p2.tile([128, 512], F32, tag="o1", name="o1")
                        nc.scalar.activation(
                            o1[:], op[:], mybir.ActivationFunctionType.Identity,
                            bias=bc2_sb[:, mch:mch + 1],
                        )
                        nc.vector.tensor_add(o1[:], o1[:], xT4[mch][:])
                        nc.sync.dma_start(
                            out=outT[bass.ts(mch, 128), bass.ts(it, 512)],
                            in_=o1[:],
                        )

                # bin after which pass-2 iteration `it` becomes ready
                p2_after = {}
                for it in range(NT2 // 4):
                    need = min(NB, -(-((it + 1) * 512) // CPB))
                    p2_after.setdefault(need - 1, []).append(it)

                kvg = None
                for lb in range(NB):
                    if lb % GB == 0:
                        nbin = min(GB, NB - lb)
                        nidx = nbin * B * 128
                        ic0 = lb * B * 8
                        kvg = p1g.tile(
                            [128, GB * B * 2 * EMBED], BF16, tag="kvg",
                            name=f"kvg{lb}",
                        )
                        nc.gpsimd.dma_gather(
                            kvg[:].rearrange(
                                "p (b n) -> p b n", n=2 * EMBED
                            )[:, 0:nbin * B, :],
                            kv_cat[:, :],
                            idxf_sb[:, ic0:ic0 + nbin * B * 8],
                            num_idxs=nidx, num_idxs_reg=nidx,
                            elem_size=2 * EMBED, single_packet=False,
                        )
                    kvv = kvg[:].rearrange("p (b n) -> p b n", n=2 * EMBED)
                    boff = (lb % GB) * B

                    st_sb = p1.tile([CPB, B * 128], BF16, tag="st", name="st")
                    nc.sync.dma_start(
                        out=st_sb[:],
                        in_=ST_in[:, lb * B * 128:(lb + 1) * B * 128],
                    )
                    s_sb = p1.tile([128, B * CPB], BF16, tag="s", name="s")
                    nc.scalar.dma_start(
                        out=s_sb[:], in_=S_in[:, lb * B * CPB:(lb + 1) * B * CPB]
                    )
                    qc_sb = p1.tile([CPB, EMBED], BF16, tag="qc", name="qc")
                    nc.scalar.dma_start(
                        out=qc_sb[:], in_=qproj[lb * CPB:(lb + 1) * CPB, :]
                    )

                    ebin = p1.tile([128, B * HEADS], F32, tag="ebin", name="ebin")
                    for j0 in range(0, B, 3):
                        g = min(3, B - j0)
                        qg_ps = p1qs.tile(
                            [128, g * EMBED], F32, tag="qg", name=f"qg{lb}_{j0}"
                        )
                        for j in range(j0, j0 + g):
                            nc.tensor.matmul(
                                qg_ps[:, bass.ts(j - j0, EMBED)],
                                st_sb[:, bass.ts(j, 128)], qc_sb[:],
                                start=True, stop=True,
                            )
                        prod = p1.tile(
                            [128, g * EMBED], BF16, tag="prod",
                            name=f"prod{lb}_{j0}",
                        )
                        nc.vector.tensor_mul(
                            prod[:].rearrange("p (b n) -> p b n", n=EMBED),
                            kvv[:, boff + j0:boff + j0 + g, 0:EMBED],
                            qg_ps[:].rearrange("p (b n) -> p b n", n=EMBED),
                        )
                        nc.vector.reduce_sum(
                            ebin[:, j0 * HEADS:(j0 + g) * HEADS]
                            .rearrange("p (o h) -> p o h", o=1),
                            prod[:].rearrange("p (h d) -> p h d", d=HD),
                            axis=mybir.AxisListType.X,
                        )
                    EXT = EMBED + HEADS
                    pvb = p1.tile([128, B * EXT], BF16, tag="pv", name=f"pv{lb}")
                    pvbv = pvb[:].rearrange("p (b n) -> p b n", n=EXT)
                    nc.scalar.activation(
                        pvbv[:, :, EMBED:EXT],
                        ebin[:].rearrange("p (b h) -> p b h", h=HEADS),
                        mybir.ActivationFunctionType.Exp,
                    )
                    oc_ps = p1ps.tile([CPB, EXT], F32, tag="oc", name="oc")
                    for j0 in range(0, B, 3):
                        g = min(3, B - j0)
                        nc.vector.tensor_mul(
                            pvbv[:, j0:j0 + g, 0:EMBED]
                            .rearrange("p b (h d) -> p b h d", d=HD),
                            kvv[:, boff + j0:boff + j0 + g, EMBED:2 * EMBED]
                            .rearrange("p b (h d) -> p b h d", d=HD),
                            pvbv[:, j0:j0 + g, EMBED:EXT][:, :, :, None]
                            .to_broadcast([128, g, HEADS, HD]),
                        )
                    for j in range(B):
                        nc.tensor.matmul(
                            oc_ps[:], s_sb[:, bass.ts(j, CPB)],
                            pvb[:, bass.ts(j, EXT)],
                            start=(j == 0), stop=(j == B - 1),
                        )
                    dn = p1.tile([CPB, HEADS], F32, tag="dnsb", name="dnsb")
                    nc.vector.tensor_scalar_add(
                        dn[:], oc_ps[:, EMBED:EXT], 1e-30
                    )
                    rcp = p1.tile([CPB, HEADS], F32, tag="rcp", name="rcp")
                    nc.vector.reciprocal(rcp[:], dn[:])
                    an = p1.tile([CPB, EMBED], BF16, tag="an", name="an")
                    nc.vector.tensor_mul(
                        an[:].rearrange("p (h d) -> p h d", d=HD),
                        oc_ps[:, 0:EMBED].rearrange("p (h d) -> p h d", d=HD),
                        rcp[:][:, :, None].to_broadcast([CPB, HEADS, HD]),
                    )
                    nc.sync.dma_start(
                        out=attn[lb * CPB:(lb + 1) * CPB, :], in_=an[:]
                    )
                    for it in p2_after.get(lb, []):
                        emit_pass2(it)

            # ---- pass 2: (interleaved above) ----
    nc.compile()
    return nc


def kernel(**inputs):
    in_maps, cell_of_slot, B = _host_prep(inputs)
    if B not in _PROG_CACHE:
        _PROG_CACHE[B] = _build_program(B)
    nc = _PROG_CACHE[B]
    res = bass_utils.run_bass_kernel_spmd(nc, in_maps, core_ids=list(range(NCORES)))
    out = np.zeros((TGT, EMBED), np.float32)
    for c in range(NCORES):
        oc = res.results[c]["outT"].T  # [4096, 256]
        mask = cell_of_slot[c] >= 0
        out[cell_of_slot[c][mask]] = oc[mask]
    return out
